# revision 8
# baseline (speedup 1.0000x reference)
"""Diffusion stencil kernel for Trainium2 (8 NeuronCores).

Problem: 10 iterations of x += c*(grad0(x)+grad1(x)+grad2(x)) on a
(64, 1024, 1024) fp32 volume, torch.gradient semantics (central diffs
interior, one-sided at boundaries), c = ALPHA*DT = 0.05.

Design:
- Shard axis1 (1024) across 8 cores, 128 rows each. Full inputs are
  staged per-core with a 5-row axis1 halo, so no collectives: the kernel
  runs as 2 launches of a K=5 fused-iteration program, with host-side
  resharding between launches.
- SBUF layout: partitions = (a2-block pair j) x (a0=64); free dims =
  (a1 patch 138, a2 patch 42). Two a2-blocks of 32 columns (each with a
  5-col halo) ride in the two partition halves of every tile.
- Per level: TensorE does 5 float32r matmul passes into PSUM:
  block-diag tridiagonal (axis0 gradient incl. one-sided boundary rows)
  plus 4 shifted-window identity passes (+/-a1, +/-a2, scaled c/2).
  VectorE then does ONE fused scalar_tensor_tensor per chunk:
  out = (state * 1.0) + psum -- the identity add stays exact fp32.
  ScalarE casts state -> float32r copy (matmul operands must be f32r-
  rounded). GpSimd rebuilds boundary ghost rows/cols each level
  (x[-1] := 2x[0]-x[1] makes the central diff equal the one-sided diff).
"""
import numpy as np

NUM_ITERATIONS = 10
C = 0.5 * 0.1          # ALPHA * DT
CG = C * 0.5

D0, D1, D2 = 64, 1024, 1024
NCORES = 8
SH1 = D1 // NCORES     # 128 rows of axis1 per core
K = 5                  # fused iterations per launch
S2 = 32                # a2 columns owned per block
W2 = S2 + 2 * K        # 42 patch cols
W1 = SH1 + 2 * K       # 138 patch rows
NBLK = D2 // S2        # 32 blocks
NPAIR = NBLK // 2      # 16 pairs
D2P = D2 + 2 * K       # padded a2 extent (1034)

_cache = {}


def _build_matrices():
    # T64[q, m] = weight of input a0-row q in output a0-row m (gradient only,
    # no identity), scaled by C.  One-sided at global a0 boundaries.
    t = np.zeros((64, 64), dtype=np.float32)
    for m in range(64):
        if m == 0:
            t[0, 0] = -C
            t[1, 0] = C
        elif m == 63:
            t[62, 63] = -C
            t[63, 63] = C
        else:
            t[m - 1, m] = -CG
            t[m + 1, m] = CG
    wtri = np.zeros((128, 128), dtype=np.float32)
    wtri[:64, :64] = t
    wtri[64:, 64:] = t
    wp = np.eye(128, dtype=np.float32) * CG
    wm = np.eye(128, dtype=np.float32) * -CG
    return wtri, wp, wm


def _build_program():
    import concourse.tile as tile
    from concourse import bacc, mybir

    f32 = mybir.dt.float32
    f32r = mybir.dt.float32r
    ALU = mybir.AluOpType

    nc = bacc.Bacc(None)
    xin = nc.declare_dram_parameter("xin", [NBLK, D0, W1, W2], f32, isOutput=False)
    wtri_in = nc.declare_dram_parameter("wtri", [128, 128], f32, isOutput=False)
    wp_in = nc.declare_dram_parameter("wp", [128, 128], f32, isOutput=False)
    wm_in = nc.declare_dram_parameter("wm", [128, 128], f32, isOutput=False)
    mlo_in = nc.declare_dram_parameter("mlo", [128, 1], f32, isOutput=False)
    mhi_in = nc.declare_dram_parameter("mhi", [128, 1], f32, isOutput=False)
    xout = nc.declare_dram_parameter("xout", [NBLK, D0, SH1, S2], f32, isOutput=True)

    with tile.TileContext(nc) as tc:
        with (
            tc.tile_pool(name="wpool", bufs=1) as wpool,
            tc.tile_pool(name="state", bufs=5) as state_pool,
            tc.tile_pool(name="crp", bufs=2) as cr_pool,
            tc.tile_pool(name="gtmp", bufs=2) as gtmp_pool,
            tc.tile_pool(name="psum", bufs=8, space="PSUM") as psum_pool,
        ):
            # --- constants: DMA in, cast weights to f32r on ACT ---
            wtri_f = wpool.tile([128, 128], f32, tag="wtri_f")
            wp_f = wpool.tile([128, 128], f32, tag="wp_f")
            wm_f = wpool.tile([128, 128], f32, tag="wm_f")
            nc.sync.dma_start(wtri_f[:], wtri_in[:])
            nc.sync.dma_start(wp_f[:], wp_in[:])
            nc.sync.dma_start(wm_f[:], wm_in[:])
            wtri = wpool.tile([128, 128], f32r, tag="wtri")
            wp = wpool.tile([128, 128], f32r, tag="wp")
            wm = wpool.tile([128, 128], f32r, tag="wm")
            nc.scalar.copy(wtri[:], wtri_f[:])
            nc.scalar.copy(wp[:], wp_f[:])
            nc.scalar.copy(wm[:], wm_f[:])
            mlo = wpool.tile([128, 1], f32, tag="mlo")
            mhi = wpool.tile([128, 1], f32, tag="mhi")
            nc.sync.dma_start(mlo[:], mlo_in[:])
            nc.sync.dma_start(mhi[:], mhi_in[:])

            for p in range(NPAIR):
                st = state_pool.tile([128, W1, W2], f32, tag="st")
                nc.sync.dma_start(st[0:64, :, :], xin[2 * p])
                nc.sync.dma_start(st[64:128, :, :], xin[2 * p + 1])

                for t in range(K):
                    rv0, rv1 = t + 1, W1 - 1 - t     # output row range
                    cv0, cv1 = t + 1, W2 - 1 - t     # output col range
                    gc0, gc1 = t, W2 - t             # ghost-row col window
                    gr0, gr1 = t, W1 - t             # ghost-col row window

                    # --- ghost rows (a1 global edges; per-core mask blend) ---
                    dlo = gtmp_pool.tile([128, 1, W2], f32, tag="g0")
                    nc.vector.scalar_tensor_tensor(
                        dlo[:, :, gc0:gc1], st[:, 5:6, gc0:gc1], 2.0,
                        st[:, 6:7, gc0:gc1], op0=ALU.mult, op1=ALU.subtract)
                    elo = gtmp_pool.tile([128, 1, W2], f32, tag="g1")
                    nc.vector.scalar_tensor_tensor(
                        elo[:, :, gc0:gc1], st[:, 4:5, gc0:gc1], -1.0,
                        dlo[:, :, gc0:gc1], op0=ALU.mult, op1=ALU.add)
                    nc.vector.scalar_tensor_tensor(
                        st[:, 4:5, gc0:gc1], elo[:, :, gc0:gc1], mlo[:, 0:1],
                        st[:, 4:5, gc0:gc1], op0=ALU.mult, op1=ALU.add)
                    dhi = gtmp_pool.tile([128, 1, W2], f32, tag="g2")
                    nc.vector.scalar_tensor_tensor(
                        dhi[:, :, gc0:gc1], st[:, W1 - 6:W1 - 5, gc0:gc1], 2.0,
                        st[:, W1 - 7:W1 - 6, gc0:gc1], op0=ALU.mult, op1=ALU.subtract)
                    ehi = gtmp_pool.tile([128, 1, W2], f32, tag="g3")
                    nc.vector.scalar_tensor_tensor(
                        ehi[:, :, gc0:gc1], st[:, W1 - 5:W1 - 4, gc0:gc1], -1.0,
                        dhi[:, :, gc0:gc1], op0=ALU.mult, op1=ALU.add)
                    nc.vector.scalar_tensor_tensor(
                        st[:, W1 - 5:W1 - 4, gc0:gc1], ehi[:, :, gc0:gc1],
                        mhi[:, 0:1], st[:, W1 - 5:W1 - 4, gc0:gc1],
                        op0=ALU.mult, op1=ALU.add)
                    # --- ghost cols (a2 global edges; static blocks 0/31) ---
                    if p == 0:
                        nc.vector.scalar_tensor_tensor(
                            st[0:64, gr0:gr1, 4:5], st[0:64, gr0:gr1, 5:6], 2.0,
                            st[0:64, gr0:gr1, 6:7], op0=ALU.mult, op1=ALU.subtract)
                    if p == NPAIR - 1:
                        nc.vector.scalar_tensor_tensor(
                            st[64:128, gr0:gr1, W2 - 5:W2 - 4],
                            st[64:128, gr0:gr1, W2 - 6:W2 - 5], 2.0,
                            st[64:128, gr0:gr1, W2 - 7:W2 - 6],
                            op0=ALU.mult, op1=ALU.subtract)

                    # --- cast state -> f32r for matmul consumption (ACT) ---
                    cr = cr_pool.tile([128, W1, W2], f32r, tag="cr")
                    nc.scalar.copy(cr[:, gr0:gr1, gc0:gc1], st[:, gr0:gr1, gc0:gc1])

                    stn = state_pool.tile([128, W1, W2], f32, tag="st")
                    ncols = cv1 - cv0
                    dr_max = 512 // ncols
                    r0 = rv0
                    while r0 < rv1:
                        dr = min(dr_max, rv1 - r0)
                        ps = psum_pool.tile([128, dr, ncols], f32, tag="ps")
                        nc.tensor.matmul(
                            ps[:], wtri[:], cr[:, r0:r0 + dr, cv0:cv1],
                            start=True, stop=False)
                        nc.tensor.matmul(
                            ps[:], wp[:], cr[:, r0 + 1:r0 + dr + 1, cv0:cv1],
                            start=False, stop=False)
                        nc.tensor.matmul(
                            ps[:], wm[:], cr[:, r0 - 1:r0 + dr - 1, cv0:cv1],
                            start=False, stop=False)
                        nc.tensor.matmul(
                            ps[:], wp[:], cr[:, r0:r0 + dr, cv0 + 1:cv1 + 1],
                            start=False, stop=False)
                        nc.tensor.matmul(
                            ps[:], wm[:], cr[:, r0:r0 + dr, cv0 - 1:cv1 - 1],
                            start=False, stop=True)
                        nc.vector.scalar_tensor_tensor(
                            stn[:, r0:r0 + dr, cv0:cv1],
                            st[:, r0:r0 + dr, cv0:cv1], 1.0, ps[:],
                            op0=ALU.mult, op1=ALU.add)
                        r0 += dr
                    st = stn

                nc.sync.dma_start(
                    xout[2 * p], st[0:64, K:K + SH1, K:K + S2])
                nc.sync.dma_start(
                    xout[2 * p + 1], st[64:128, K:K + SH1, K:K + S2])

    nc.finalize()
    return nc


def _stage_inputs(xfull):
    """Per-core, per-block contiguous input tiles (NBLK, D0, W1, W2)."""
    wtri, wp, wm = _cache["mats"]
    in_maps = []
    for c in range(NCORES):
        slab = np.zeros((D0, W1, D2P), dtype=np.float32)
        r0 = c * SH1 - K
        rlo = max(r0, 0)
        rhi = min(c * SH1 + SH1 + K, D1)
        slab[:, rlo - r0:rhi - r0, K:K + D2] = xfull[:, rlo:rhi, :]
        xt = np.empty((NBLK, D0, W1, W2), dtype=np.float32)
        for b in range(NBLK):
            xt[b] = slab[:, :, b * S2:b * S2 + W2]
        in_maps.append({
            "xin": xt,
            "wtri": wtri, "wp": wp, "wm": wm,
            "mlo": np.full((128, 1), 1.0 if c == 0 else 0.0, np.float32),
            "mhi": np.full((128, 1), 1.0 if c == NCORES - 1 else 0.0, np.float32),
        })
    return in_maps


def _run_pass(xfull, trace=False):
    from concourse.bass_utils import run_bass_kernel_spmd
    nc = _cache["nc"]
    res = run_bass_kernel_spmd(nc, _stage_inputs(xfull),
                               core_ids=list(range(NCORES)), trace=trace)
    # xout per core: (NBLK, D0, SH1, S2) -> (D0, SH1, D2)
    cores = [res.results[c]["xout"].transpose(1, 2, 0, 3).reshape(D0, SH1, D2)
             for c in range(NCORES)]
    out = np.concatenate(cores, axis=1)
    return out, res.exec_time_ns


def kernel(x):
    x = np.asarray(x, dtype=np.float32)
    if "nc" not in _cache:
        _cache["mats"] = _build_matrices()
        _cache["nc"] = _build_program()
    mid, t1 = _run_pass(x)
    out, t2 = _run_pass(mid)
    _cache["exec_time_ns"] = (t1 or 0) + (t2 or 0)
    return out
